# revision 1
# baseline (speedup 1.0000x reference)
"""Trainium2 Bass kernel for FMGCNCell (adaptive-graph GRU cell), v3 (fp16).

Per-call host->device staging dominates (~0.53 ms/MB/core), so each core
ships only its own node slab (~3 MB) and the full X matrix is assembled
on-device via AllGather. The whole datapath runs in fp16 (10-bit mantissa)
with f32 PSUM accumulation; the adaptive support is pre-normalized so its
fp16 copy stays in [0, 1]. Output returns fp16, host converts.

Sharding: node-parallel (each core owns N/8 = 250 output nodes, full batch).
"""

from contextlib import ExitStack

import numpy as np
import ml_dtypes

import concourse.bass as bass
import concourse.bacc as bacc
import concourse.tile as tile
from concourse import mybir
from concourse.bass import ds, ts
from concourse.bass_utils import run_bass_kernel_spmd
from concourse.masks import make_identity

F32 = mybir.dt.float32
BF16 = mybir.dt.bfloat16
F16 = mybir.dt.float16
AF = mybir.ActivationFunctionType
ALU = mybir.AluOpType

B = 64
DIN = 2
H = 64
E = 16
CAT = DIN + H            # 66
KI0 = CAT + 1            # 67
KIALL = 2 * CAT + 1      # 133
OG = 2 * H               # 128
OU = H                   # 64
BC = B * CAT             # 4224; X layout (c, b) c-major
N = 2000
NC_ = 8
NOWN = N // NC_          # 250
NB = B * NOWN            # 16000


def _chunks(total, size):
    out = []
    off = 0
    while off < total:
        out.append((off, min(size, total - off)))
        off += size
    return out


def build_nc(n_cores=8, nblk=25):
    mch = _chunks(N, 128)
    MC = len(mch)

    nc = bacc.Bacc("TRN2", target_bir_lowering=False, debug=False,
                   num_devices=n_cores)

    # ---- external inputs ----
    xt_d = nc.dram_tensor("xt_in", [KI0, NB], F16, kind="ExternalInput")
    eT_all_d = nc.dram_tensor("eT_all", [E, N], F32, kind="ExternalInput")
    eT_own_d = nc.dram_tensor("eT_own", [E, NOWN], F32, kind="ExternalInput")
    wpg_d = nc.dram_tensor("wpg", [E, KIALL, OG], F16, kind="ExternalInput")
    wpu_d = nc.dram_tensor("wpu", [E, KIALL, OU], F16, kind="ExternalInput")

    # output: [h, (n, b)] n-major, fp16 (host reassembles)
    out_d = nc.dram_tensor("out", [H, NB], F16, kind="ExternalOutput")

    # ---- internal DRAM ----
    x1_own_d = nc.dram_tensor("x1_own", [NOWN, BC], F16)
    zs_own_d = nc.dram_tensor("zs_own", [NOWN, H * B], F16)
    wg_d = nc.dram_tensor("wg_d", [KIALL, NOWN, OG], F16)
    wu_d = nc.dram_tensor("wu_d", [KIALL, NOWN, OU], F16)
    xg_d = nc.dram_tensor("xg_d", [BC, NOWN], F16)
    r_d = nc.dram_tensor("r_d", [NOWN, H * B], F16)
    shared = dict(addr_space="Shared") if n_cores > 1 else {}
    x1_all_d = nc.dram_tensor("x1_all", [N, BC], F16, **shared)
    zs_all_d = nc.dram_tensor("zs_all", [N, H * B], F16, **shared)

    with tile.TileContext(nc) as tc:
        with ExitStack() as root:
            persist = root.enter_context(tc.tile_pool(name="persist", bufs=1))
            XT = persist.tile([KI0, NB], F16)            # [c, (b, n)] b-major
            A_sb = persist.tile([128, MC * NOWN], F16)   # normalized support
            xcols = persist.tile([128, MC * 128], F16)   # X cols 4096:4224
            XG = persist.tile([CAT, NB], F16)            # xg1T then xg2T
            ident = persist.tile([128, 128], F16)
            make_identity(nc, ident[:])
            XB = persist.tile([3, NB], F16)              # x|ones rows at part 0
            eT_own = persist.tile([E, NOWN], F32)
            eT16 = persist.tile([E, NOWN], F16)

            # ---------- P0: load inputs ----------
            nc.sync.dma_start(XT[:], xt_d[:])
            nc.sync.dma_start(XB[:], xt_d[H:KI0, :])
            nc.sync.dma_start(eT_own[:], eT_own_d[:])
            nc.vector.tensor_copy(eT16[:], eT_own[:])

            # ---------- P2: build x1_own from XT by transposes ----------
            with ExitStack() as p2:
                tps = p2.enter_context(tc.tile_pool(name="tps", bufs=3, space="PSUM"))
                stg = p2.enter_context(tc.tile_pool(name="stg", bufs=1))
                S0 = stg.tile([128, BC], F16)
                S1 = stg.tile([128, BC], F16)
                for b in range(B):
                    for half, S in ((0, S0), (1, S1)):
                        ps = tps.tile([128, 128], F16, tag="tps")
                        nc.tensor.transpose(
                            ps[:125, :CAT],
                            XT[:CAT, b * NOWN + half * 125:
                               b * NOWN + half * 125 + 125],
                            ident[:CAT, :CAT])
                        nc.vector.tensor_copy(
                            S[:125, :].rearrange("n (c b2) -> n c b2", b2=B)[:, :, b],
                            ps[:125, :CAT])
                nc.sync.dma_start(x1_own_d[:125, :], S0[:125, :])
                nc.sync.dma_start(x1_own_d[125:, :], S1[:125, :])

            # ---------- P3: AllGather x1 ----------
            if n_cores > 1:
                nc.gpsimd.collective_compute(
                    "AllGather", ALU.bypass,
                    replica_groups=[list(range(n_cores))],
                    ins=[x1_own_d[:]], outs=[x1_all_d[:]])
            else:
                nc.sync.dma_start(x1_all_d[:], x1_own_d[:])

            # ---------- P1: adaptive support, normalized to fp16 ----------
            with ExitStack() as p1:
                eT_pool = p1.enter_context(tc.tile_pool(name="eT", bufs=1))
                eT_all = eT_pool.tile([E, N], F32)
                nc.sync.dma_start(eT_all[:], eT_all_d[:])
                M_pool = p1.enter_context(tc.tile_pool(name="Msb", bufs=1))
                M_sb = M_pool.tile([128, MC * NOWN], BF16)
                rinv_bc = M_pool.tile([128, NOWN], F32)
                sm_pool = p1.enter_context(tc.tile_pool(name="sm", bufs=3))
                sm_psum = p1.enter_context(tc.tile_pool(name="sm_ps", bufs=2, space="PSUM"))
                for j, (m0, mp) in enumerate(mch):
                    ps = sm_psum.tile([128, NOWN], F32, tag="sm_ps")
                    nc.tensor.matmul(ps[:mp, :], eT_all[:, m0:m0 + mp],
                                     eT_own[:, :], start=True, stop=True)
                    ex = sm_pool.tile([128, NOWN], F32, tag="sm_ex")
                    nc.scalar.activation(ex[:mp, :], ps[:mp, :], AF.Exp)
                    nc.vector.tensor_scalar_max(M_sb[:mp, ts(j, NOWN)], ex[:mp, :],
                                                1.0)
                ones_pool = p1.enter_context(tc.tile_pool(name="ones", bufs=1))
                ones = ones_pool.tile([128, 1], BF16)
                nc.vector.memset(ones[:], 1.0)
                rs_ps = sm_psum.tile([1, NOWN], F32, tag="rs_ps")
                for j, (m0, mp) in enumerate(mch):
                    nc.tensor.matmul(rs_ps[:, :], ones[:mp, :],
                                     M_sb[:mp, ts(j, NOWN)],
                                     start=(j == 0), stop=(j == MC - 1))
                rinv = sm_pool.tile([1, NOWN], F32, tag="rinv")
                nc.vector.reciprocal(rinv[:, :], rs_ps[:, :])
                nc.gpsimd.partition_broadcast(rinv_bc[:], rinv[:, :])
                for j, (m0, mp) in enumerate(mch):
                    nc.vector.tensor_tensor(A_sb[:mp, ts(j, NOWN)],
                                            M_sb[:mp, ts(j, NOWN)],
                                            rinv_bc[:mp, :], ALU.mult)

            # ---------- P3b: W-gen (fp16) -> DRAM ----------
            nch = _chunks(NOWN, 128)
            with ExitStack() as p3:
                wgen_rhs = p3.enter_context(tc.tile_pool(name="wg_rhs", bufs=3))
                wgen_ps = p3.enter_context(tc.tile_pool(name="wg_ps", bufs=2, space="PSUM"))
                wgen_pool = p3.enter_context(tc.tile_pool(name="wg_ev", bufs=3))
                for (wdram_in, wdram, O) in ((wpg_d, wg_d, OG), (wpu_d, wu_d, OU)):
                    KO = KIALL * O
                    for (f0, fp) in _chunks(KO, 512):
                        rhs = wgen_rhs.tile([E, 512], F16, tag="wg_rhs")
                        nc.sync.dma_start(
                            rhs[:, :fp],
                            wdram_in[:].rearrange("e k o -> e (k o)")[:, f0:f0 + fp])
                        for (nn0, np_) in nch:
                            ps = wgen_ps.tile([128, 512], F32, tag="wg_ps")
                            nc.tensor.matmul(ps[:np_, :fp],
                                             eT16[:, nn0:nn0 + np_],
                                             rhs[:, :fp],
                                             start=True, stop=True)
                            ev = wgen_pool.tile([128, 512], F16, tag="wg_ev")
                            nc.vector.tensor_copy(ev[:np_, :fp], ps[:np_, :fp])
                            nki = fp // O
                            nc.sync.dma_start(
                                wdram[f0 // O:f0 // O + nki,
                                      nn0:nn0 + np_, :]
                                .rearrange("k n o -> n k o"),
                                ev[:np_, :fp]
                                .rearrange("n (k o) -> n k o", o=O))

            # ---------- conv helper: A @ X -> xg_d -> XG ----------
            def conv(parts, is_first):
                with ExitStack() as pc:
                    xg_pool = pc.enter_context(tc.tile_pool(name="xgrp", bufs=2))
                    cv_ps = pc.enter_context(tc.tile_pool(name="cv_ps", bufs=5, space="PSUM"))
                    cv_ev = pc.enter_context(tc.tile_pool(name="cv_ev", bufs=3))

                    def emit_q(q, lhs_tile, qi, gcols):
                        ps = cv_ps.tile([128, NOWN], F32, tag="cv_ps")
                        for j, (m0, mp) in enumerate(mch):
                            nc.tensor.matmul(
                                ps[:128, :],
                                lhs_tile[:mp, ts(j, gcols)][:, qi * 128:(qi + 1) * 128]
                                if gcols else lhs_tile[:mp, ts(j, 128)],
                                A_sb[:mp, ts(j, NOWN)],
                                start=(j == 0), stop=(j == MC - 1))
                        ev = cv_ev.tile([128, NOWN], F16, tag="cv_ev")
                        nc.vector.tensor_copy(ev[:, :], ps[:, :])
                        nc.sync.dma_start(xg_d[q * 128:(q + 1) * 128, :],
                                          ev[:, :])

                    for (dram, g0, gcols, qoffs) in parts:
                        if dram is None:
                            emit_q(qoffs[0], xcols, 0, 0)
                            continue
                        Xg = xg_pool.tile([128, MC * 5 * 128], F16, tag="xgrp")
                        for j, (m0, mp) in enumerate(mch):
                            nc.sync.dma_start(Xg[:mp, ts(j, gcols)],
                                              dram[m0:m0 + mp, g0:g0 + gcols])
                            if is_first and g0 + gcols == BC:
                                nc.vector.tensor_copy(
                                    xcols[:mp, ts(j, 128)],
                                    Xg[:mp, ts(j, gcols)][:, gcols - 128:])
                        for qi, q in enumerate(qoffs):
                            emit_q(q, Xg, qi, gcols)
                    nc.sync.dma_start(
                        XG[:].rearrange("c (b2 n) -> c b2 n", b2=B),
                        xg_d[:].rearrange("(c b2) n -> c b2 n", b2=B))

            def groups(q0, nq, cap=5):
                out = []
                q = q0
                while q < q0 + nq:
                    take = min(cap, q0 + nq - q)
                    out.append(list(range(q, q + take)))
                    q += take
                return out

            # ---------- P4: conv1 -> XG = xg1T ----------
            parts1 = [(x1_all_d, qs[0] * 128, len(qs) * 128, qs)
                      for qs in groups(0, 33)]
            conv(parts1, is_first=True)

            # ---------- P5: apply gate; spill z*state and r ----------
            with ExitStack() as p5:
                ap_w = p5.enter_context(tc.tile_pool(name="ap_w", bufs=2))
                ap_ps = p5.enter_context(tc.tile_pool(name="ap_ps", bufs=3, space="PSUM"))
                ap_t = p5.enter_context(tc.tile_pool(name="ap_t", bufs=3))
                for (nb0, nbp) in _chunks(NOWN, nblk):
                    w0 = ap_w.tile([KI0, nblk * OG], F16, tag="w0")
                    nc.sync.dma_start(
                        w0[:, :nbp * OG].rearrange("k (n o) -> k n o", o=OG),
                        wg_d[:KI0, nb0:nb0 + nbp, :])
                    w1 = ap_w.tile([CAT, nblk * OG], F16, tag="w1")
                    nc.sync.dma_start(
                        w1[:, :nbp * OG].rearrange("k (n o) -> k n o", o=OG),
                        wg_d[KI0:, nb0:nb0 + nbp, :])
                    for (g0, gp) in _chunks(nbp, 8):
                        ps_z = ap_ps.tile([H, 512], F32, tag="ap_psz")
                        ps_r = ap_ps.tile([H, 512], F32, tag="ap_psr")
                        for nl in range(g0, g0 + gp):
                            n = nb0 + nl
                            w = (nl - g0) * B
                            xt_n = XT[:, n::NOWN][:, :B]
                            xg_n = XG[:, n::NOWN][:, :B]
                            for ps, o0 in ((ps_z, 0), (ps_r, H)):
                                sl = slice(nl * OG + o0, nl * OG + o0 + H)
                                nc.tensor.matmul(ps[:, w:w + B], w0[:, sl],
                                                 xt_n, start=True, stop=False)
                                nc.tensor.matmul(ps[:, w:w + B], w1[:CAT, sl],
                                                 xg_n, start=False, stop=True)
                        cols = slice((nb0 + g0) * B, (nb0 + g0 + gp) * B)
                        zg = ap_t.tile([H, 512], F32, tag="zg")
                        nc.scalar.activation(zg[:, :gp * B], ps_z[:, :gp * B],
                                             AF.Sigmoid)
                        rg = ap_t.tile([H, 512], F16, tag="rg")
                        nc.scalar.activation(rg[:, :gp * B], ps_r[:, :gp * B],
                                             AF.Sigmoid)
                        nc.sync.dma_start(
                            r_d[nb0 + g0:nb0 + g0 + gp, :]
                            .rearrange("n (h b2) -> h n b2", b2=B),
                            rg[:, :gp * B].rearrange("h (n b2) -> h n b2", b2=B))
                        zs = ap_t.tile([H, 512], F16, tag="zs")
                        nc.vector.tensor_tensor(
                            zs[:, :gp * B].rearrange("h (n b2) -> h n b2", b2=B),
                            zg[:, :gp * B].rearrange("h (n b2) -> h n b2", b2=B),
                            XT[:H, :].rearrange("h (b2 n) -> h n b2", b2=B)
                            [:, nb0 + g0:nb0 + g0 + gp, :],
                            ALU.mult)
                        nc.sync.dma_start(
                            zs_own_d[nb0 + g0:nb0 + g0 + gp, :]
                            .rearrange("n (h b2) -> h n b2", b2=B),
                            zs[:, :gp * B].rearrange("h (n b2) -> h n b2", b2=B))

            # ---------- P6: AllGather z*state ----------
            if n_cores > 1:
                nc.gpsimd.collective_compute(
                    "AllGather", ALU.bypass,
                    replica_groups=[list(range(n_cores))],
                    ins=[zs_own_d[:]], outs=[zs_all_d[:]])
            else:
                nc.sync.dma_start(zs_all_d[:], zs_own_d[:])

            # ---------- P7: conv2 -> XG = xg2T ----------
            parts2 = [(zs_all_d, qs[0] * 128, len(qs) * 128, qs)
                      for qs in groups(0, 32)]
            parts2.append((None, 32 * 128, 128, [32]))
            conv(parts2, is_first=False)

            # ---------- P8: apply update; blend; output ----------
            with ExitStack() as p8:
                ap_w = p8.enter_context(tc.tile_pool(name="ap_w2", bufs=2))
                ap_ps = p8.enter_context(tc.tile_pool(name="ap_ps2", bufs=3, space="PSUM"))
                ap_t = p8.enter_context(tc.tile_pool(name="ap_t2", bufs=3))
                zst_pool = p8.enter_context(tc.tile_pool(name="zstb", bufs=2))
                for (nb0, nbp) in _chunks(NOWN, nblk):
                    w0a = ap_w.tile([H, nblk * OU], F16, tag="w0a")
                    nc.sync.dma_start(
                        w0a[:, :nbp * OU].rearrange("k (n o) -> k n o", o=OU),
                        wu_d[:H, nb0:nb0 + nbp, :])
                    w0b = ap_w.tile([3, nblk * OU], F16, tag="w0b")
                    nc.sync.dma_start(
                        w0b[:, :nbp * OU].rearrange("k (n o) -> k n o", o=OU),
                        wu_d[H:KI0, nb0:nb0 + nbp, :])
                    w1 = ap_w.tile([CAT, nblk * OU], F16, tag="w1u")
                    nc.sync.dma_start(
                        w1[:, :nbp * OU].rearrange("k (n o) -> k n o", o=OU),
                        wu_d[KI0:, nb0:nb0 + nbp, :])
                    zst = zst_pool.tile([H, nblk * B], F16, tag="zstb")
                    nc.sync.dma_start(
                        zst[:, :nbp * B].rearrange("h (n b2) -> h n b2", b2=B),
                        zs_own_d[nb0:nb0 + nbp, :]
                        .rearrange("n (h b2) -> h n b2", b2=B))
                    rb = zst_pool.tile([H, nblk * B], F16, tag="rb")
                    nc.sync.dma_start(
                        rb[:, :nbp * B].rearrange("h (n b2) -> h n b2", b2=B),
                        r_d[nb0:nb0 + nbp, :]
                        .rearrange("n (h b2) -> h n b2", b2=B))
                    for (g0, gp) in _chunks(nbp, 8):
                        ps = ap_ps.tile([H, 512], F32, tag="ap_ps2")
                        for nl in range(g0, g0 + gp):
                            n = nb0 + nl
                            w = (nl - g0) * B
                            sl = slice(nl * OU, (nl + 1) * OU)
                            nc.tensor.matmul(ps[:, w:w + B], w0a[:, sl],
                                             zst[:, nl * B:(nl + 1) * B],
                                             start=True, stop=False)
                            nc.tensor.matmul(ps[:, w:w + B], w0b[:, sl],
                                             XB[:, n::NOWN][:, :B],
                                             start=False, stop=False)
                            nc.tensor.matmul(ps[:, w:w + B], w1[:CAT, sl],
                                             XG[:, n::NOWN][:, :B],
                                             start=False, stop=True)
                        cols = slice((nb0 + g0) * B, (nb0 + g0 + gp) * B)
                        hc = ap_t.tile([H, 512], F32, tag="hc")
                        nc.scalar.activation(hc[:, :gp * B], ps[:, :gp * B],
                                             AF.Tanh)
                        # out = hc + r*(state - hc)
                        t1 = ap_t.tile([H, 512], F32, tag="t1")
                        nc.vector.tensor_sub(
                            t1[:, :gp * B].rearrange("h (n b2) -> h n b2", b2=B),
                            XT[:H, :].rearrange("h (b2 n) -> h n b2", b2=B)
                            [:, nb0 + g0:nb0 + g0 + gp, :],
                            hc[:, :gp * B].rearrange("h (n b2) -> h n b2", b2=B))
                        t2 = ap_t.tile([H, 512], F32, tag="t2")
                        nc.vector.tensor_tensor(
                            t2[:, :gp * B], t1[:, :gp * B],
                            rb[:, cols.start - nb0 * B:cols.stop - nb0 * B],
                            ALU.mult)
                        ot = ap_t.tile([H, 512], F16, tag="ot")
                        nc.vector.tensor_add(ot[:, :gp * B], t2[:, :gp * B],
                                             hc[:, :gp * B])
                        nc.sync.dma_start(out_d[:, cols], ot[:, :gp * B])

    nc.compile()
    return nc


_NC_CACHE = {}


def _get_nc(n_cores=8):
    if n_cores not in _NC_CACHE:
        _NC_CACHE[n_cores] = build_nc(n_cores=n_cores)
    return _NC_CACHE[n_cores]


def _pack_pool(wp, bias, O):
    """[E,K,CAT,O] pool + [E,O] bias -> [E, 133, O] fp16.

    Row order per k-slab: (state rows, x rows); bias at row 66."""
    out = np.empty((E, KIALL, O), np.float32)
    out[:, :H, :] = wp[:, 0, DIN:, :]
    out[:, H:CAT, :] = wp[:, 0, :DIN, :]
    out[:, CAT, :] = bias
    out[:, KI0:KI0 + H, :] = wp[:, 1, DIN:, :]
    out[:, KI0 + H:, :] = wp[:, 1, :DIN, :]
    return out.astype(np.float16)


def _build_in_maps(x, state, node_embed, gate_weights_pool, gate_bias_pool,
                   update_weights_pool, update_bias_pool, n_cores=8):
    x = np.asarray(x, np.float32)
    state = np.asarray(state, np.float32)
    node_embed = np.asarray(node_embed, np.float32)
    eT = np.ascontiguousarray(node_embed.T)                 # [E, N]
    x1_nbc = np.concatenate([state.transpose(1, 0, 2), x.transpose(1, 0, 2)],
                            axis=2)                          # [N, B, CAT]
    wpg = _pack_pool(np.asarray(gate_weights_pool, np.float32),
                     np.asarray(gate_bias_pool, np.float32), OG)
    wpu = _pack_pool(np.asarray(update_weights_pool, np.float32),
                     np.asarray(update_bias_pool, np.float32), OU)
    in_maps = []
    for c in range(n_cores):
        sl = slice(c * NOWN, (c + 1) * NOWN)
        eT_own = np.ascontiguousarray(eT[:, sl])
        xt = np.ones((KI0, B, NOWN), np.float32)
        xt[:CAT] = x1_nbc[sl].transpose(2, 1, 0)
        in_maps.append({
            "xt_in": xt.astype(np.float16).reshape(KI0, NB),
            "eT_all": eT,
            "eT_own": eT_own,
            "wpg": wpg,
            "wpu": wpu,
        })
    return in_maps


def kernel(x, state, node_embed, gate_weights_pool, gate_bias_pool,
           update_weights_pool, update_bias_pool, n_cores=8):
    nc = _get_nc(n_cores)
    in_maps = _build_in_maps(x, state, node_embed, gate_weights_pool,
                             gate_bias_pool, update_weights_pool,
                             update_bias_pool, n_cores)
    res = run_bass_kernel_spmd(nc, in_maps, list(range(n_cores)))
    outs = []
    for c in range(n_cores):
        o = np.asarray(res.results[c]["out"], dtype=np.float32)  # [H, NOWN*B]
        outs.append(o.reshape(H, NOWN, B).transpose(2, 1, 0))    # [B, NOWN, H]
    return np.concatenate(outs, axis=1)



# revision 15
# speedup vs baseline: 1.1494x; 1.1494x over previous
"""Trainium2 Bass kernel for FMGCNCell (adaptive-graph GRU cell), v4.

Node-parallel across 8 cores (250 nodes each), full batch per core.
Datapath fp16 with f32 PSUM accumulation.

v4 vs v3: host pre-packs x1_own so AllGather #1 fires at t=0; per-node
weights generated TRANSPOSED (c on partitions) straight into SBUF (no W
DRAM roundtrip); gate z/r fused into one [128,64] matmul per node;
contraction split 127 + 6 with the 6-row remainder as one E-linearized
grouped matmul (shared stationary WpB, rhs XhatB built by
replicate+scale); zs AllGather split in halves; r spilled contiguously;
update weight-gen hidden in the AllGather window.
"""

from contextlib import ExitStack

import numpy as np

import concourse.bass as bass
import concourse.bacc as bacc
import concourse.tile as tile
from concourse import mybir
from concourse.bass_utils import run_bass_kernel_spmd

F32 = mybir.dt.float32
BF16 = mybir.dt.bfloat16
F16 = mybir.dt.float16
AF = mybir.ActivationFunctionType
ALU = mybir.AluOpType

B = 64
DIN = 2
H = 64
E = 16
CAT = DIN + H            # 66
KI0 = CAT + 1            # 67  (state, x, ones)
KIALL = 2 * CAT + 1      # 133 (k0: state,x,ones | k1: state,x)
OG = 2 * H               # 128
OU = H                   # 64
N = 2000
NC_ = 8
NOWN = N // NC_          # 250
NB = B * NOWN            # 16000
BC = B * CAT             # 4224 x1 row length, (c-major, b-minor)
MCH = 125                # m-chunk (aligns rank halves)
MC = N // MCH            # 16
NH = NOWN // 2           # 125 nodes per AG half
CA = 127                 # per-node contraction rows (0..126)
CB = KIALL - CA          # 6 remainder rows (127..132)
DI = E * CB              # 96 linearized (d, c) rows
ZCOL = H * B             # 4096 zs columns in conv2 space


def build_nc(n_cores=8):
    nc = bacc.Bacc("TRN2", target_bir_lowering=False, debug=False,
                   num_devices=n_cores)

    # ---- external inputs ----
    xt_d = nc.dram_tensor("xt_in", [KI0, NB], F16, kind="ExternalInput")
    x1_own_d = nc.dram_tensor("x1_own", [NOWN, BC], F16, kind="ExternalInput")
    eT_all_d = nc.dram_tensor("eT_all", [E, N], F16, kind="ExternalInput")
    eT_own_d = nc.dram_tensor("eT_own", [E, NOWN], F16, kind="ExternalInput")
    wpo_g_d = nc.dram_tensor("wpo_g", [E, OG * KIALL], F16,
                             kind="ExternalInput")
    wpo_u_d = nc.dram_tensor("wpo_u", [E, OU * KIALL], F16,
                             kind="ExternalInput")
    wpB_g_d = nc.dram_tensor("wpB_g", [DI, OG], F16, kind="ExternalInput")
    wpB_u_d = nc.dram_tensor("wpB_u", [DI, OU], F16, kind="ExternalInput")
    ebn_d = nc.dram_tensor("ebn", [DI, NOWN], F16, kind="ExternalInput")

    out_d = nc.dram_tensor("out", [H, NB], F16, kind="ExternalOutput")

    # ---- internal DRAM ----
    x1_int_d = nc.dram_tensor("x1_int", [NOWN, BC], F16)
    xg_d = nc.dram_tensor("xg_scratch", [33 * 128, NOWN], F16)
    zs_a_d = nc.dram_tensor("zs_a", [NH, ZCOL], F16)
    zs_b_d = nc.dram_tensor("zs_b", [NH, ZCOL], F16)
    r_d = nc.dram_tensor("r_spill", [H, NB], F16)
    shared = dict(addr_space="Shared") if n_cores > 1 else {}
    x1_all_d = nc.dram_tensor("x1_all", [N, BC], F16, **shared)
    zsa_all_d = nc.dram_tensor("zsa_all", [NC_ * NH, ZCOL], F16, **shared)
    zsb_all_d = nc.dram_tensor("zsb_all", [NC_ * NH, ZCOL], F16, **shared)

    with tile.TileContext(nc) as tc:
        with ExitStack() as root:
            persist = root.enter_context(tc.tile_pool(name="persist", bufs=1))
            XA = persist.tile([128, NB], F16)       # apply lhs rows 0..126
            XhatB = persist.tile([DI, NB], F16)     # linearized remainder rhs
            A_sb = persist.tile([MCH, MC * NOWN], F16)
            WT = persist.tile([128, OG * NOWN], F16)  # W^T rows 0..126
            WpB = persist.tile([DI, OG], F16)
            Ebn = persist.tile([DI, NOWN], F16)
            eT_own = persist.tile([E, NOWN], F16)

            # ---------- t=0: stage x1, fire AllGather #1 ----------
            nc.sync.dma_start(x1_int_d[:], x1_own_d[:])
            if n_cores > 1:
                nc.gpsimd.collective_compute(
                    "AllGather", ALU.bypass,
                    replica_groups=[list(range(n_cores))],
                    ins=[x1_int_d[:]], outs=[x1_all_d[:]])
            else:
                nc.sync.dma_start(x1_all_d[:], x1_int_d[:])

            # ---------- small loads ----------
            nc.sync.dma_start(XA[:KI0, :], xt_d[:])
            nc.sync.dma_start(eT_own[:], eT_own_d[:])
            nc.sync.dma_start(WpB[:], wpB_g_d[:])
            nc.sync.dma_start(Ebn[:], ebn_d[:])

            # ---------- adaptive support A^T [m, n_own] fp16 ----------
            with ExitStack() as p1:
                ab = p1.enter_context(tc.tile_pool(name="ab", bufs=1))
                eT_all = ab.tile([E, N], F16)
                nc.sync.dma_start(eT_all[:], eT_all_d[:])
                M_sb = ab.tile([MCH, MC * NOWN], BF16)
                rinv_bc = ab.tile([MCH, NOWN], F32)
                ones = ab.tile([MCH, 1], BF16)
                nc.vector.memset(ones[:], 1.0)
                sm_ps = p1.enter_context(
                    tc.tile_pool(name="sm_ps", bufs=3, space="PSUM"))
                sm_ev = p1.enter_context(tc.tile_pool(name="sm_ev", bufs=3))
                for j in range(MC):
                    ps = sm_ps.tile([MCH, NOWN], F32, tag="sm_ps")
                    nc.tensor.matmul(ps[:], eT_all[:, j * MCH:(j + 1) * MCH],
                                     eT_own[:], start=True, stop=True)
                    ex = sm_ev.tile([MCH, NOWN], F32, tag="sm_ex")
                    nc.scalar.activation(ex[:], ps[:], AF.Exp)
                    # exp(relu(s)) = max(exp(s), 1)
                    nc.vector.tensor_scalar_max(
                        M_sb[:, j * NOWN:(j + 1) * NOWN], ex[:], 1.0)
                rs_ps = sm_ps.tile([1, NOWN], F32, tag="rs_ps")
                for j in range(MC):
                    nc.tensor.matmul(rs_ps[:], ones[:],
                                     M_sb[:, j * NOWN:(j + 1) * NOWN],
                                     start=(j == 0), stop=(j == MC - 1))
                rinv = sm_ev.tile([1, NOWN], F32, tag="rinv")
                nc.vector.reciprocal(rinv[:], rs_ps[:])
                nc.gpsimd.partition_broadcast(rinv_bc[:], rinv[:])
                for j in range(MC):
                    nc.vector.tensor_tensor(
                        A_sb[:, j * NOWN:(j + 1) * NOWN],
                        M_sb[:, j * NOWN:(j + 1) * NOWN],
                        rinv_bc[:], ALU.mult)

            # ---------- W^T gen: WT[0:127, o*NOWN:(o+1)*NOWN] ----------
            def wt_gen(wpo_dram, O):
                with ExitStack() as pw:
                    wp_pool = pw.enter_context(tc.tile_pool(name="wp", bufs=2))
                    wg_ps = pw.enter_context(
                        tc.tile_pool(name="wg_ps", bufs=3, space="PSUM"))
                    for o0 in range(0, O, 16):
                        wp = wp_pool.tile([E, 16 * KIALL], F16, tag="wp")
                        nc.sync.dma_start(
                            wp[:], wpo_dram[:, o0 * KIALL:(o0 + 16) * KIALL])
                        for oi in range(16):
                            o = o0 + oi
                            ps = wg_ps.tile([128, NOWN], F32, tag="wg_ps")
                            nc.tensor.matmul(
                                ps[:CA, :], wp[:, oi * KIALL:oi * KIALL + CA],
                                eT_own[:], start=True, stop=True)
                            nc.vector.tensor_copy(
                                WT[:CA, o * NOWN:(o + 1) * NOWN], ps[:CA, :])

            wt_gen(wpo_g_d, OG)

            # ---------- conv: xg = (A @ X)^T, scattered to XA/XhatB ------
            # col c of X maps to: c<=59 -> XA row 67+c ; else XhatB row c-60
            def conv(load_src):
                """load_src(j, col0, ncols) -> (dram_ap) for X columns
                [col0, col0+ncols) of m-chunk j in (c b) flat space."""
                with ExitStack() as pc:
                    xg_pool = pc.enter_context(
                        tc.tile_pool(name="xgl", bufs=3))
                    cv_ps = pc.enter_context(
                        tc.tile_pool(name="cv_ps", bufs=6, space="PSUM"))
                    cv_ev = pc.enter_context(
                        tc.tile_pool(name="cv_ev", bufs=4))
                    for qq0 in range(0, 33, 5):
                        qn = min(5, 33 - qq0)
                        pss = [cv_ps.tile([128, NOWN], F32, tag="cv_ps",
                                          name=f"cvps{qq0}_{k}")
                               for k in range(qn)]
                        for j in range(MC):
                            Xg = xg_pool.tile([MCH, 5 * 128], F16, tag="xgl")
                            for (dram_ap, x0, ncols) in load_src(
                                    j, qq0 * 128, qn * 128):
                                eng = nc.scalar if j % 2 else nc.sync
                                eng.dma_start(Xg[:, x0:x0 + ncols], dram_ap)
                            for qi in range(qn):
                                nc.tensor.matmul(
                                    pss[qi][:],
                                    Xg[:, qi * 128:(qi + 1) * 128],
                                    A_sb[:, j * NOWN:(j + 1) * NOWN],
                                    start=(j == 0), stop=(j == MC - 1))
                        for qi in range(qn):
                            q = qq0 + qi
                            ev = cv_ev.tile([128, NOWN], F16, tag="cv_ev")
                            nc.vector.tensor_copy(ev[:], pss[qi][:])
                            nc.scalar.dma_start(
                                xg_d[q * 128:(q + 1) * 128, :], ev[:])
                            c0 = 2 * q
                            if c0 <= 58:
                                dst = XA[67 + c0:69 + c0, :]
                            else:
                                dst = XhatB[c0 - 60:c0 - 58, :]
                            eng2 = nc.sync if q % 2 else nc.scalar
                            eng2.dma_start(
                                dst.rearrange("c (b n) -> c b n", n=NOWN),
                                xg_d[q * 128:(q + 1) * 128, :]
                                .rearrange("(c b) n -> c b n", b=B))

            def src_conv1(j, col0, ncols):
                return [(x1_all_d[j * MCH:(j + 1) * MCH, col0:col0 + ncols],
                         0, ncols)]

            conv(src_conv1)

            # ---------- XhatB: replicate rows 0..5 over d, scale by E ------
            def xhatb_finish():
                for d in range(1, E):
                    nc.sync.dma_start(XhatB[d * CB:(d + 1) * CB, :],
                                      XhatB[:CB, :])
                nc.vector.tensor_tensor(
                    XhatB[:].rearrange("p (b n) -> p b n", n=NOWN),
                    XhatB[:].rearrange("p (b n) -> p b n", n=NOWN),
                    Ebn[:, None, :].broadcast_to([DI, B, NOWN]),
                    ALU.mult)

            xhatb_finish()

            # ---------- gate apply ----------
            WTv = WT[:].rearrange("c (o n) -> c o n", n=NOWN)
            XAv = XA[:].rearrange("c (b n) -> c b n", n=NOWN)
            XBv = XhatB[:].rearrange("c (b n) -> c n b", n=NOWN)
            with ExitStack() as p5:
                zs_pool = p5.enter_context(tc.tile_pool(name="zsp", bufs=1))
                zs_sb = zs_pool.tile([H, NB], F16)
                ap_ps = p5.enter_context(
                    tc.tile_pool(name="ap_ps", bufs=3, space="PSUM"))
                ap_t = p5.enter_context(tc.tile_pool(name="ap_t", bufs=3))
                for g0 in range(0, NOWN, 8):
                    gn = min(8, NOWN - g0)
                    ps = ap_ps.tile([OG, 8 * B], F32, tag="ap_ps")
                    nc.tensor.matmul(ps[:, :gn * B],
                                     WpB[:, :OG],
                                     XBv[:, g0:g0 + gn, :],
                                     start=True, stop=False)
                    for nl in range(gn):
                        n = g0 + nl
                        nc.tensor.matmul(ps[:, nl * B:(nl + 1) * B],
                                         WTv[:CA, :OG, n], XAv[:CA, :, n],
                                         start=False, stop=(nl == gn - 1))
                    sig = ap_t.tile([OG, 8 * B], F16, tag="sig")
                    nc.scalar.activation(sig[:, :gn * B], ps[:, :gn * B],
                                         AF.Sigmoid)
                    nc.sync.dma_start(r_d[:, g0 * B:(g0 + gn) * B],
                                      sig[H:, :gn * B])
                    nc.vector.tensor_tensor(
                        zs_sb[:, g0 * B:(g0 + gn) * B]
                        .rearrange("h (n b) -> h n b", b=B),
                        sig[:H, :gn * B].rearrange("h (n b) -> h n b", b=B),
                        XAv[:H, :, g0:g0 + gn].rearrange("h b n -> h n b"),
                        ALU.mult)
                    # AG payload [n, (h b)]; split a group straddling NH
                    segs = []
                    if g0 + gn <= NH or g0 >= NH:
                        segs.append((g0, gn))
                    else:
                        segs.append((g0, NH - g0))
                        segs.append((NH, g0 + gn - NH))
                    for (s0, sn) in segs:
                        half_d = zs_a_d if s0 < NH else zs_b_d
                        hb = s0 if s0 < NH else s0 - NH
                        nc.gpsimd.dma_start(
                            half_d[hb:hb + sn, :]
                            .rearrange("n (h b) -> h n b", b=B),
                            zs_sb[:, s0 * B:(s0 + sn) * B]
                            .rearrange("h (n b) -> h n b", b=B))

                # ---------- AllGather #2 (two halves) ----------
                if n_cores > 1:
                    nc.gpsimd.collective_compute(
                        "AllGather", ALU.bypass,
                        replica_groups=[list(range(n_cores))],
                        ins=[zs_a_d[:]], outs=[zsa_all_d[:]])
                    nc.gpsimd.collective_compute(
                        "AllGather", ALU.bypass,
                        replica_groups=[list(range(n_cores))],
                        ins=[zs_b_d[:]], outs=[zsb_all_d[:]])
                else:
                    nc.sync.dma_start(zsa_all_d[:], zs_a_d[:])
                    nc.sync.dma_start(zsb_all_d[:], zs_b_d[:])

                # update apply matrix rows 0-63 <- zs (frees zs_sb after)
                nc.vector.tensor_copy(
                    XA[:H, :].rearrange("h (b n) -> h n b", n=NOWN),
                    zs_sb[:].rearrange("h (n b) -> h n b", b=B))

            # ---------- AG window: update W^T, WpB, XTs ----------
            wt_gen(wpo_u_d, OU)
            nc.sync.dma_start(WpB[:, :OU], wpB_u_d[:])
            upd = root.enter_context(tc.tile_pool(name="upd", bufs=1))
            XTs = upd.tile([H, NB], F16)
            nc.sync.dma_start(XTs[:], xt_d[:H, :])

            # ---------- conv2 over [zs_all | x cols of x1_all] ----------
            def src_conv2(j, col0, ncols):
                rank, half = j // 2, j % 2
                zsrc = zsa_all_d if half == 0 else zsb_all_d
                out = []
                nz = max(0, min(ncols, ZCOL - col0))
                if nz > 0:
                    out.append((zsrc[rank * NH:(rank + 1) * NH,
                                     col0:col0 + nz], 0, nz))
                if ncols > nz:
                    out.append((x1_all_d[j * MCH:(j + 1) * MCH,
                                         col0 + nz:col0 + ncols], nz,
                                ncols - nz))
                return out

            conv(src_conv2)
            xhatb_finish()

            # ---------- update apply + blend + output ----------
            with ExitStack() as p8:
                ap_ps = p8.enter_context(
                    tc.tile_pool(name="ap_ps2", bufs=3, space="PSUM"))
                ap_t = p8.enter_context(tc.tile_pool(name="ap_t2", bufs=2))
                XTv = XTs[:].rearrange("h (b n) -> h n b", n=NOWN)
                for g0 in range(0, NOWN, 8):
                    gn = min(8, NOWN - g0)
                    gb = gn * B
                    ps = ap_ps.tile([OU, 8 * B], F32, tag="ap_ps2")
                    nc.tensor.matmul(ps[:, :gb], WpB[:, :OU],
                                     XBv[:, g0:g0 + gn, :],
                                     start=True, stop=False)
                    for nl in range(gn):
                        n = g0 + nl
                        nc.tensor.matmul(ps[:, nl * B:(nl + 1) * B],
                                         WTv[:CA, :OU, n], XAv[:CA, :, n],
                                         start=False, stop=(nl == gn - 1))
                    hc = ap_t.tile([OU, 8 * B], F32, tag="hc")
                    nc.scalar.activation(hc[:, :gb], ps[:, :gb], AF.Tanh)
                    rb = ap_t.tile([OU, 8 * B], F16, tag="rb")
                    nc.sync.dma_start(rb[:, :gb],
                                      r_d[:, g0 * B:(g0 + gn) * B])
                    # out = hc + r*(state - hc)
                    t1 = ap_t.tile([OU, 8 * B], F32, tag="t1")
                    nc.vector.tensor_sub(
                        t1[:, :gb].rearrange("h (n b) -> h n b", b=B),
                        XTv[:, g0:g0 + gn, :],
                        hc[:, :gb].rearrange("h (n b) -> h n b", b=B))
                    t2 = ap_t.tile([OU, 8 * B], F32, tag="t2")
                    nc.vector.tensor_tensor(t2[:, :gb], t1[:, :gb],
                                            rb[:, :gb], ALU.mult)
                    ot = ap_t.tile([OU, 8 * B], F16, tag="ot")
                    nc.vector.tensor_add(ot[:, :gb], t2[:, :gb], hc[:, :gb])
                    nc.sync.dma_start(out_d[:, g0 * B:(g0 + gn) * B],
                                      ot[:, :gb])

    nc.compile()
    return nc


_NC_CACHE = {}


def _get_nc(n_cores=8):
    if n_cores not in _NC_CACHE:
        _NC_CACHE[n_cores] = build_nc(n_cores=n_cores)
    return _NC_CACHE[n_cores]


def _pack_pool(wp, bias, O):
    """[E,K,CAT,O] pool + [E,O] bias -> packed [E, 133, O] f32.

    Row order per k-slab: (state rows, x rows); bias at row 66."""
    out = np.empty((E, KIALL, O), np.float32)
    out[:, :H, :] = wp[:, 0, DIN:, :]
    out[:, H:CAT, :] = wp[:, 0, :DIN, :]
    out[:, CAT, :] = bias
    out[:, KI0:KI0 + H, :] = wp[:, 1, DIN:, :]
    out[:, KI0 + H:, :] = wp[:, 1, :DIN, :]
    return out


def _build_in_maps(x, state, node_embed, gate_weights_pool, gate_bias_pool,
                   update_weights_pool, update_bias_pool, n_cores=8):
    x = np.asarray(x, np.float32)
    state = np.asarray(state, np.float32)
    node_embed = np.asarray(node_embed, np.float32)
    eT = np.ascontiguousarray(node_embed.T).astype(np.float16)   # [E, N]
    x1_nbc = np.concatenate([state.transpose(1, 0, 2), x.transpose(1, 0, 2)],
                            axis=2)                               # [N, B, CAT]
    pg = _pack_pool(np.asarray(gate_weights_pool, np.float32),
                    np.asarray(gate_bias_pool, np.float32), OG)
    pu = _pack_pool(np.asarray(update_weights_pool, np.float32),
                    np.asarray(update_bias_pool, np.float32), OU)
    wpo_g = np.ascontiguousarray(pg.transpose(0, 2, 1)).reshape(
        E, OG * KIALL).astype(np.float16)
    wpo_u = np.ascontiguousarray(pu.transpose(0, 2, 1)).reshape(
        E, OU * KIALL).astype(np.float16)
    wpB_g = np.ascontiguousarray(pg[:, CA:, :].reshape(DI, OG)).astype(
        np.float16)
    wpB_u = np.ascontiguousarray(pu[:, CA:, :].reshape(DI, OU)).astype(
        np.float16)
    in_maps = []
    for c in range(n_cores):
        sl = slice(c * NOWN, (c + 1) * NOWN)
        xt = np.ones((KI0, B, NOWN), np.float32)
        xt[:CAT] = x1_nbc[sl].transpose(2, 1, 0)
        x1o = np.ascontiguousarray(
            x1_nbc[sl].transpose(0, 2, 1)).reshape(NOWN, BC)
        eT_own_c = np.ascontiguousarray(eT[:, sl])
        ebn = np.ascontiguousarray(
            np.repeat(eT_own_c, CB, axis=0))              # [DI, NOWN]
        in_maps.append({
            "xt_in": xt.astype(np.float16).reshape(KI0, NB),
            "x1_own": x1o.astype(np.float16),
            "eT_all": eT,
            "eT_own": eT_own_c,
            "wpo_g": wpo_g,
            "wpo_u": wpo_u,
            "wpB_g": wpB_g,
            "wpB_u": wpB_u,
            "ebn": ebn,
        })
    return in_maps


def kernel(x, state, node_embed, gate_weights_pool, gate_bias_pool,
           update_weights_pool, update_bias_pool, n_cores=8):
    nc = _get_nc(n_cores)
    in_maps = _build_in_maps(x, state, node_embed, gate_weights_pool,
                             gate_bias_pool, update_weights_pool,
                             update_bias_pool, n_cores)
    res = run_bass_kernel_spmd(nc, in_maps, list(range(n_cores)))
    outs = []
    for c in range(n_cores):
        o = np.asarray(res.results[c]["out"], dtype=np.float32)  # [H, NB]
        outs.append(o.reshape(H, NOWN, B).transpose(2, 1, 0))    # [B,NOWN,H]
    return np.concatenate(outs, axis=1)
